# revision 3
# baseline (speedup 1.0000x reference)
"""BiCGSTAB (4 iters, 7-point stencil) on 8 NeuronCores — fp16/fused version.

Problem: x,b,ref: [2,256,256,256] f32, center: [1,256,256,1] f32.
Output: final x after 4 BiCGSTAB iterations (f32).

Sharding: core c handles batch c//4, H-slab 64*(c%4)..+64. Dot products are
4-rank AllReduces in groups [[0..3],[4..7]]; H-halo ghost planes are filled
from AllGathers via indirect-DMA gathers (edge cores read a zeroed row
range, implementing the Dirichlet boundary).

vs the f32 baseline:
- Fields stored fp16 (x,b pre-converted host-side); v stored as v/4 and the
  stencil applied with coefficients scaled by 1/4 (center reaches 262, so
  S(S(.)) overflows fp16 unscaled; sim'd rel err 1.4e-3 vs 2e-2 gate).
- The whole stencil (center via per-plane diag matrices, W/H/Z shifts) runs
  on the TensorEngine as accumulating fp16 matmuls into PSUM; ScalarE
  copies PSUM->SBUF with scale factors folded in.
- Two fused sweeps per iteration instead of 5 field passes:
    sweep1: s = r - 4a*v~; t' = S4(s); dots <t',s>,<t',t'>,<r0,t'>; store s
            (s boundary planes exchanged under the omega-AllReduce barrier).
    sweep2: recompute t' from s; r' = s - 4w*t'; p' = r' + B*(p - 4w*v~);
            v~' = S4(p'); x += a*p + w*s; dot <r0,v~'>;
            r/v/p boundary planes exchanged for the next iteration.
"""
import numpy as np

import concourse.bacc as bacc
import concourse.bass as bass
import concourse.bass_isa as bass_isa
import concourse.mybir as mybir
import concourse.tile as tile

F32 = mybir.dt.float32
F16 = mybir.dt.float16
I32 = mybir.dt.int32

N_CORES = 8
GROUP = 4
EPS = 1e-6
SC = 16.0   # stencil coefficient scale: psum = S(u)/SC
# halo_in slots: 0:r[0] 1:r[HC-1] 2:v[0] 3:v[HC-1] 4:p[0] 5:p[HC-1]
#                6:s[0] 7:s[1] 8:s[HC-2] 9:s[HC-1]
NSLOT = 10


def _fills_rvp(HC):
    """(field, buffer_row, neighbor_slot, from_left); r/v/p are [HC+2]."""
    return [
        ("r", 0, 1, True), ("r", HC + 1, 0, False),
        ("v", 0, 3, True), ("v", HC + 1, 2, False),
        ("p", 0, 5, True), ("p", HC + 1, 4, False),
    ]


def _fills_s(HC):
    """s is [HC+4] with +-2 ghosts."""
    return [
        ("s", 0, 8, True), ("s", 1, 9, True),
        ("s", HC + 2, 6, False), ("s", HC + 3, 7, False),
    ]


def build_program(HC=64, W=256, Z=256, KH=8, ITERS=4, collectives=True,
                  twin_reps=0, dump=None, dump_iters=None,
                  nodots=False, nostencil=False, noupd=False, nofill=False,
                  diag_const=True, nbufs=3):
    assert W == 256 and Z == 256
    NB = HC // KH
    RG = [list(range(GROUP)), list(range(GROUP, 2 * GROUP))]
    ZR = GROUP * NSLOT * W
    PPQ = 2  # planes per 512-col psum chunk

    twin = twin_reps > 0
    assert not (twin and collectives)
    nc = bacc.Bacc("TRN2", target_bir_lowering=False, debug=False,
                   num_devices=N_CORES)

    if twin:
        x_in = nc.dram_tensor("x16_t", [HC + 4, W, Z], F16)
        b_in = nc.dram_tensor("b16_t", [HC + 2, W, Z], F16)
        x_out = nc.dram_tensor("xout_t", [HC, W, Z], F32)
        dummy_out = nc.dram_tensor("dummy_o", [1, 8], F32,
                                   kind="ExternalOutput")
    else:
        x_in = nc.dram_tensor("x16", [HC + 4, W, Z], F16,
                              kind="ExternalInput")
        b_in = nc.dram_tensor("b16", [HC + 2, W, Z], F16,
                              kind="ExternalInput")
        x_out = nc.dram_tensor("xout", [HC, W, Z], F32, kind="ExternalOutput")
    mats_in = nc.dram_tensor("mats", [128, 4 * 128], F16, kind="ExternalInput")
    NDM = 2 if diag_const else 2 * (HC + 2)
    dmats_in = nc.dram_tensor("dmats", [128, NDM * 128], F16,
                              kind="ExternalInput")
    idx_in = nc.dram_tensor("idx", [2 * 128, NSLOT], I32, kind="ExternalInput")
    medge_in = nc.dram_tensor("medge", [128, 2], F32, kind="ExternalInput")

    with tile.TileContext(nc) as tc:
        with (
            tc.tile_pool(name="sb", bufs=2) as sb,
            tc.tile_pool(name="ps", bufs=8, space="PSUM") as ps,
            tc.tile_pool(name="dr", bufs=1, space="DRAM") as dr,
        ):
            _cnt = [0]

            def _nm(pfx):
                _cnt[0] += 1
                return f"{pfx}{_cnt[0]}"

            # ---- DRAM intermediates
            r0_d = dr.tile([HC, W, Z], F16, tag="r0")
            s_d = dr.tile([HC + 4, W, Z], F16, tag="s")
            rr = [dr.tile([HC + 2, W, Z], F16, tag=f"r{i}", name=f"fld_r{i}")
                  for i in range(2)]
            vv = [dr.tile([HC + 2, W, Z], F16, tag=f"v{i}", name=f"fld_v{i}")
                  for i in range(2)]
            pp = [dr.tile([HC + 2, W, Z], F16, tag=f"p{i}", name=f"fld_p{i}")
                  for i in range(2)]
            xx = [dr.tile([HC, W, Z], F16, tag=f"x{i}", name=f"fld_x{i}")
                  for i in range(2)]
            halo_in = dr.tile([NSLOT * W, Z], F16, tag="halo_in")
            halo_out = dr.tile([ZR + 128, Z], F16, tag="halo_out")
            din = [dr.tile([1, 8], F32, tag=f"din{i}", name=f"din_{i}")
                   for i in range(3)]
            dout = [dr.tile([1, 8], F32, tag=f"dout{i}", name=f"dout_{i}")
                    for i in range(3)]

            # ---- persistent SBUF constants
            mats_sb = sb.tile([128, 4 * 128], F16, tag="mats", bufs=1)
            nc.sync.dma_start(out=mats_sb[:], in_=mats_in[:, :])
            An = mats_sb[:, 0:128]
            Bn = [mats_sb[:, 128:256], mats_sb[:, 256:384]]
            In = mats_sb[:, 384:512]
            dmats_sb = sb.tile([128, NDM * 128], F16, tag="dmats",
                               bufs=1)
            nc.sync.dma_start(out=dmats_sb[:], in_=dmats_in[:, :])

            def Dg(wc, h):
                k = wc if diag_const else wc * (HC + 2) + h + 1
                return dmats_sb[:, k * 128:(k + 1) * 128]

            idx_sb = []
            for wc in range(2):
                it_ = sb.tile([128, NSLOT], I32, tag=f"idx{wc}", bufs=1,
                              name=f"idx_{wc}")
                nc.sync.dma_start(out=it_[:],
                                  in_=idx_in[wc * 128:(wc + 1) * 128, :])
                idx_sb.append(it_)
            medge = sb.tile([128, 2], F32, tag="medge", bufs=1)
            nc.sync.dma_start(out=medge[:], in_=medge_in[:, :])

            zt = sb.tile([128, Z], F16, tag="gh")
            nc.vector.memset(zt[:], 0.0)
            nc.sync.dma_start(out=halo_out[ZR:ZR + 128, :], in_=zt[:])
            z8 = sb.tile([1, 8], F32, tag="z8", bufs=1)
            nc.vector.memset(z8[:], 0.0)
            for i in range(3):
                nc.sync.dma_start(out=din[i][:, :], in_=z8[:])

            # ---- helpers ------------------------------------------------
            def stage(src_plane_ap, slot, wc):
                r_ = slot * W + wc * 128
                nc.sync.dma_start(out=halo_in[r_:r_ + 128, :],
                                  in_=src_plane_ap)

            def allgather():
                if collectives:
                    nc.gpsimd.collective_compute(
                        "AllGather", mybir.AluOpType.bypass, replica_groups=RG,
                        ins=[halo_in[:, :].opt()],
                        outs=[halo_out[0:ZR, :].opt()])

            def fill(fills, flds, icol0):
                if nofill:
                    return
                for i, (f, row, _s, _l) in enumerate(fills):
                    for wc in range(2):
                        g = sb.tile([128, Z], F16, tag="gh", name=_nm("gh"))
                        nc.gpsimd.indirect_dma_start(
                            out=g[:], out_offset=None, in_=halo_out[:, :],
                            in_offset=bass.IndirectOffsetOnAxis(
                                ap=idx_sb[wc][:, icol0 + i:icol0 + i + 1],
                                axis=0))
                        nc.sync.dma_start(
                            out=flds[f][row, wc * 128:wc * 128 + 128, :],
                            in_=g[:])

            def fill_rvp(si):
                fill(_fills_rvp(HC), {"r": rr[si], "v": vv[si], "p": pp[si]},
                     0)

            def fill_s():
                fill(_fills_s(HC), {"s": s_d}, 6)

            def load_win(field, wc, npl, row0, tag, bufs=None):
                w0 = wc * 128
                t_ = sb.tile([128, npl, Z], F16, tag=tag, name=_nm("w"),
                             bufs=bufs if bufs else nbufs)
                nc.sync.dma_start(
                    out=t_[:],
                    in_=field[row0:row0 + npl, w0:w0 + 128, :].rearrange(
                        "h w z -> w h z"))
                return t_

            def store_blk(field, src_ap, wc, npl, row0):
                w0 = wc * 128
                nc.sync.dma_start(
                    out=field[row0:row0 + npl, w0:w0 + 128, :].rearrange(
                        "h w z -> w h z"),
                    in_=src_ap)

            def stencil_apply(sw, so, wc, h0, npl, out_tile, mul=None,
                              combine=None):
                """out_tile = S4(field) on npl planes [h0..h0+npl-1].

                sw/so: this/other chunk windows covering planes
                [h0-1 .. h0+npl] (npl+2 planes). mul: [128,1] AP folded into
                the ScalarE PSUM->SBUF copy. combine(dst, pt, q): custom DVE
                combine instead.
                """
                nq = (npl * Z) // 512
                of = out_tile[:].rearrange("p h z -> p (h z)")
                if nostencil:
                    return
                for q in range(nq):
                    pt = ps.tile([128, 512], F32, tag="pt", name=_nm("pt"))
                    p3 = pt[:].rearrange("p (h z) -> p h z", h=PPQ)
                    qp = q * PPQ
                    nc.tensor.matmul(
                        out=pt[:], lhsT=An,
                        rhs=sw[:, 1 + qp:1 + qp + PPQ, :].rearrange(
                            "p h z -> p (h z)"),
                        start=True, stop=False)
                    nc.tensor.matmul(
                        out=pt[:], lhsT=Bn[wc],
                        rhs=so[:, 1 + qp:1 + qp + PPQ, :].rearrange(
                            "p h z -> p (h z)"),
                        start=False, stop=False)
                    nc.tensor.matmul(
                        out=pt[:], lhsT=In,
                        rhs=sw[:, qp:qp + PPQ, :].rearrange(
                            "p h z -> p (h z)"),
                        start=False, stop=False)
                    nc.tensor.matmul(
                        out=pt[:], lhsT=In,
                        rhs=sw[:, 2 + qp:2 + qp + PPQ, :].rearrange(
                            "p h z -> p (h z)"),
                        start=False, stop=False)
                    nc.tensor.matmul(
                        out=p3[:, :, 1:Z], lhsT=In,
                        rhs=sw[:, 1 + qp:1 + qp + PPQ, 0:Z - 1],
                        start=False, stop=False)
                    nc.tensor.matmul(
                        out=p3[:, :, 0:Z - 1], lhsT=In,
                        rhs=sw[:, 1 + qp:1 + qp + PPQ, 1:Z],
                        start=False, stop=False)
                    if diag_const:
                        nc.tensor.matmul(
                            out=pt[:], lhsT=Dg(wc, 0),
                            rhs=sw[:, 1 + qp:1 + qp + PPQ, :].rearrange(
                                "p h z -> p (h z)"),
                            start=False, stop=True)
                    else:
                        for pi in range(PPQ):
                            nc.tensor.matmul(
                                out=p3[:, pi, :], lhsT=Dg(wc, h0 + qp + pi),
                                rhs=sw[:, 1 + qp + pi, :],
                                start=False, stop=(pi == PPQ - 1))
                    dst = of[:, q * 512:(q + 1) * 512]
                    if combine is not None:
                        combine(dst, pt, q)
                    elif mul is None:
                        nc.scalar.copy(out=dst, in_=pt[:])
                    else:
                        nc.scalar.mul(out=dst, in_=pt[:], mul=mul)

            def ttr(in0, in1, acc_prev, scr, tag="accA"):
                acc = sb.tile([128, 1], F32, tag=tag + "p", bufs=4,
                              name=_nm("acc"))
                if nodots:
                    nc.vector.memset(acc[:], 1.0)
                    return acc
                nc.vector.scalar_tensor_tensor(
                    out=scr, in0=in0, scalar=1.0, in1=in1,
                    op0=mybir.AluOpType.mult, op1=mybir.AluOpType.mult,
                    accum_out=acc[:])
                if acc_prev is None:
                    return acc
                tot = sb.tile([128, 1], F32, tag=tag, bufs=4, name=_nm("accs"))
                nc.vector.tensor_add(out=tot[:], in0=acc_prev[:], in1=acc[:])
                return tot

            def finish_dot(acc, dtile, col):
                red = sb.tile([128, 1], F32, tag="dred", bufs=8, name=_nm("rd"))
                nc.gpsimd.partition_all_reduce(
                    red[:], acc[:], channels=128,
                    reduce_op=bass_isa.ReduceOp.add)
                nc.sync.dma_start(out=dtile[0:1, col:col + 1],
                                  in_=red[0:1, 0:1])

            def allreduce(i):
                if collectives:
                    nc.gpsimd.collective_compute(
                        "AllReduce", mybir.AluOpType.add, replica_groups=RG,
                        ins=[din[i][:, :].opt()], outs=[dout[i][:, :].opt()])
                dsb = sb.tile([1, 8], F32, tag="dsb", bufs=8, name=_nm("dsb"))
                nc.sync.dma_start(out=dsb[:], in_=dout[i][:, :])
                return dsb

            def s_tile():
                return sb.tile([1, 1], F32, tag="dsc", bufs=64, name=_nm("sc"))

            def s_recip_eps(a_ap, pre=1.0):
                if pre != 1.0:
                    t2 = s_tile()
                    nc.vector.tensor_scalar_mul(out=t2[:], in0=a_ap,
                                                scalar1=pre)
                    a_ap = t2[:]
                t = s_tile()
                nc.vector.tensor_scalar_add(out=t[:], in0=a_ap, scalar1=EPS)
                r_ = s_tile()
                nc.vector.reciprocal(out=r_[:], in_=t[:])
                return r_

            def s_mul(a_ap, b_ap):
                t = s_tile()
                nc.vector.tensor_tensor(out=t[:], in0=a_ap, in1=b_ap,
                                        op=mybir.AluOpType.mult)
                return t

            def s_muli(a_ap, imm):
                t = s_tile()
                nc.vector.tensor_scalar_mul(out=t[:], in0=a_ap, scalar1=imm)
                return t

            def s_sub(a_ap, b_ap):
                t = s_tile()
                nc.vector.tensor_tensor(out=t[:], in0=a_ap, in1=b_ap,
                                        op=mybir.AluOpType.subtract)
                return t

            def bcast(a_ap):
                b_ = sb.tile([128, 1], F32, tag="bc", bufs=16, name=_nm("bc"))
                nc.gpsimd.partition_broadcast(b_[:], a_ap, channels=128)
                return b_

            def stt(out, in0, sc, in1):
                nc.vector.scalar_tensor_tensor(
                    out=out, in0=in0, scalar=sc, in1=in1,
                    op0=mybir.AluOpType.mult, op1=mybir.AluOpType.add)

            from contextlib import ExitStack as _ES
            _loop = _ES()
            if twin:
                _loop.enter_context(tc.For_i(0, twin_reps, 1))

            # ================= P0 =======================================
            accR = accD = None
            for j in range(NB):
                h0 = j * KH
                xw = [load_win(x_in, wc, KH + 4, h0, f"rw{wc}")
                      for wc in range(2)]
                bw = [load_win(b_in, wc, KH + 2, h0, f"vw{wc}")
                      for wc in range(2)]
                r0w = []
                for wc in range(2):
                    t_ = sb.tile([128, KH + 2, Z], F16, tag=f"sw{wc}",
                                 name=_nm("r0w"), bufs=nbufs)
                    bf = bw[wc][:].rearrange("p h z -> p (h z)")

                    def comb(dst, pt, q, bf=bf):
                        nc.vector.scalar_tensor_tensor(
                            out=dst, in0=pt[:], scalar=-SC,
                            in1=bf[:, q * 512:(q + 1) * 512],
                            op0=mybir.AluOpType.mult, op1=mybir.AluOpType.add)

                    stencil_apply(xw[wc], xw[1 - wc], wc, h0 - 1, KH + 2, t_,
                                  combine=comb)
                    if j == 0:
                        nc.vector.tensor_scalar_mul(
                            out=t_[:, 0, :], in0=t_[:, 0, :],
                            scalar1=medge[:, 0:1])
                    if j == NB - 1:
                        nc.vector.tensor_scalar_mul(
                            out=t_[:, KH + 1, :], in0=t_[:, KH + 1, :],
                            scalar1=medge[:, 1:2])
                    r0w.append(t_)
                for wc in range(2):
                    vb = sb.tile([128, KH, Z], F16, tag="vb", name=_nm("vb"))
                    stencil_apply(r0w[wc], r0w[1 - wc], wc, h0, KH, vb)
                    r0c = r0w[wc][:, 1:KH + 1, :]
                    scr = sb.tile([128, KH + 2, Z], F16, tag="scr",
                                  name=_nm("scr"))
                    sc8 = scr[:, 0:KH, :]
                    accR = ttr(r0c, r0c, accR, sc8, "accR")
                    accD = ttr(r0c, vb[:], accD, sc8, "accD")
                    store_blk(r0_d, r0c, wc, KH, h0)
                    store_blk(rr[0], r0c, wc, KH, h0 + 1)
                    store_blk(vv[0], vb[:], wc, KH, h0 + 1)
                    if j == 0:
                        stage(r0w[wc][:, 1, :], 0, wc)
                        stage(vb[:, 0, :], 2, wc)
                        stage(r0w[wc][:, 1, :], 4, wc)
                    if j == NB - 1:
                        stage(r0w[wc][:, KH, :], 1, wc)
                        stage(vb[:, KH - 1, :], 3, wc)
                        stage(r0w[wc][:, KH, :], 5, wc)
            allgather()
            fill_rvp(0)
            finish_dot(accR, din[0], 0)
            finish_dot(accD, din[0], 1)
            dsb0 = allreduce(0)
            rho_ap = dsb0[0:1, 0:1]
            d1_ap = dsb0[0:1, 1:2]  # <r0,v~>; <r0,v> = 4*d1

            for it in range(ITERS):
                last = it == ITERS - 1
                src, dst = it % 2, 1 - it % 2
                r_src, v_src = rr[src], vv[src]
                p_src = rr[0] if it == 0 else pp[src]
                r_dst, v_dst, p_dst = rr[dst], vv[dst], pp[dst]

                alpha = s_mul(rho_ap, s_recip_eps(d1_ap, SC)[:])
                na4 = bcast(s_muli(alpha[:], -SC)[:])
                a_b = bcast(alpha[:])

                # ===== sweep 1 =====
                accTS = accTT = accRT = None
                for j in [0, NB - 1] + list(range(1, NB - 1)):
                    h0 = j * KH
                    rw = [load_win(r_src, wc, KH + 2, h0, f"rw{wc}")
                          for wc in range(2)]
                    vw = [load_win(v_src, wc, KH + 2, h0, f"vw{wc}")
                          for wc in range(2)]
                    sw = []
                    for wc in range(2):
                        st_ = sb.tile([128, KH + 2, Z], F16, tag=f"sw{wc}",
                                      name=_nm("sw"), bufs=nbufs)
                        if not noupd:
                            stt(st_[:], vw[wc][:], na4[:], rw[wc][:])
                        sw.append(st_)
                    for wc in range(2):
                        tw = sb.tile([128, KH + 2, Z], F16, tag="tw",
                                     name=_nm("tw"), bufs=2)
                        tw8 = tw[:, 0:KH, :]
                        stencil_apply(sw[wc], sw[1 - wc], wc, h0, KH, tw8)
                        scr = sb.tile([128, KH + 2, Z], F16, tag="scr",
                                      name=_nm("scr"))
                        sc8 = scr[:, 0:KH, :]
                        sc_ = sw[wc][:, 1:KH + 1, :]
                        accTS = ttr(tw8, sc_, accTS, sc8, "accTS")
                        accTT = ttr(tw8, tw8, accTT, sc8, "accTT")
                        if not last:
                            if it == 0:
                                r0b_ap = rw[wc][:, 1:KH + 1, :]
                            else:
                                r0b_ap = load_win(r0_d, wc, KH, h0, "r0b",
                                                  bufs=1)[:]
                            accRT = ttr(r0b_ap, tw8, accRT, sc8, "accRT")
                        store_blk(s_d, sc_, wc, KH, h0 + 2)
                        if j == 0:
                            stage(sw[wc][:, 1, :], 6, wc)
                            stage(sw[wc][:, 2, :], 7, wc)
                        if j == NB - 1:
                            stage(sw[wc][:, KH - 1, :], 8, wc)
                            stage(sw[wc][:, KH, :], 9, wc)
                allgather()
                fill_s()
                finish_dot(accTS, din[1], 0)
                finish_dot(accTT, din[1], 1)
                if not last:
                    finish_dot(accRT, din[1], 2)
                dsb1 = allreduce(1)
                omega = s_mul(s_muli(dsb1[0:1, 0:1], SC)[:],
                              s_recip_eps(dsb1[0:1, 1:2], SC * SC)[:])
                o_b = bcast(omega[:])
                no4 = bcast(s_muli(omega[:], -SC)[:])
                if not last:
                    rho_n = s_sub(
                        s_sub(rho_ap,
                              s_muli(s_mul(alpha[:], d1_ap)[:], SC)[:])[:],
                        s_muli(s_mul(omega[:], dsb1[0:1, 2:3])[:], SC)[:])
                    beta = s_mul(
                        s_mul(rho_n[:], s_recip_eps(rho_ap)[:])[:],
                        s_mul(alpha[:], s_recip_eps(omega[:])[:])[:])
                    b_b = bcast(beta[:])
                    rho_ap = rho_n[:]

                # ===== sweep 2 =====
                accD = None
                for j in ([0, NB - 1] + list(range(1, NB - 1))
                          if not last else list(range(NB))):
                    h0 = j * KH
                    if last:
                        for wc in range(2):
                            sb_c = load_win(s_d, wc, KH, h0 + 2, "sw0")  # nbufs default
                            pw = load_win(p_src, wc, KH, h0 + 1, "pw", bufs=2)
                            xwb = (load_win(x_in, wc, KH, h0 + 2, "xwb",
                                            bufs=1)
                                   if it == 0 else
                                   load_win(xx[src], wc, KH, h0, "xwb",
                                            bufs=1))
                            x1 = sb.tile([128, KH, Z], F16, tag="x1",
                                         name=_nm("x1"))
                            stt(x1[:], pw[:], a_b[:], xwb[:])
                            xo = sb.tile([128, KH, Z], F32, tag="xo",
                                         name=_nm("xo"), bufs=1)
                            stt(xo[:], sb_c[:], o_b[:], x1[:])
                            store_blk(x_out, xo[:], wc, KH, h0)
                        continue
                    sw4 = [load_win(s_d, wc, KH + 4, h0, f"rw{wc}")
                           for wc in range(2)]
                    pn2 = []
                    for wc in range(2):
                        twm = sb.tile([128, KH + 2, Z], F16, tag="tw",
                                      name=_nm("twm"), bufs=2)
                        stencil_apply(sw4[wc], sw4[1 - wc], wc, h0 - 1,
                                      KH + 2, twm, mul=no4[:, 0:1])
                        rn = sb.tile([128, KH + 2, Z], F16, tag="rn",
                                     name=_nm("rn"))
                        if not noupd:
                            nc.vector.tensor_add(out=rn[:],
                                                 in0=sw4[wc][:, 1:KH + 3, :],
                                                 in1=twm[:])
                        vw2 = load_win(v_src, wc, KH + 2, h0, f"vw{wc}")
                        pw = load_win(p_src, wc, KH + 2, h0, "pw", bufs=2)
                        u = sb.tile([128, KH + 2, Z], F16, tag="scr",
                                    name=_nm("u"))
                        pn = sb.tile([128, KH + 2, Z], F16, tag=f"pn{wc}",
                                     name=_nm("pn"), bufs=2)
                        if not noupd:
                            stt(u[:], vw2[:], no4[:], pw[:])
                            stt(pn[:], u[:], b_b[:], rn[:])
                        pn2.append(pn)
                        # x update
                        xwb = (load_win(x_in, wc, KH, h0 + 2, "xwb", bufs=1)
                               if it == 0 else
                               load_win(xx[src], wc, KH, h0, "xwb", bufs=1))
                        x1 = sb.tile([128, KH, Z], F16, tag="x1",
                                     name=_nm("x1"))
                        x2 = sb.tile([128, KH, Z], F16, tag="x2",
                                     name=_nm("x2"))
                        if not noupd:
                            stt(x1[:], pw[:, 1:KH + 1, :], a_b[:], xwb[:])
                            stt(x2[:], sw4[wc][:, 2:KH + 2, :], o_b[:], x1[:])
                        store_blk(xx[dst], x2[:], wc, KH, h0)
                        store_blk(r_dst, rn[:, 1:KH + 1, :], wc, KH, h0 + 1)
                        store_blk(p_dst, pn[:, 1:KH + 1, :], wc, KH, h0 + 1)
                        if j == 0:
                            stage(rn[:, 1, :], 0, wc)
                            stage(pn[:, 1, :], 4, wc)
                        if j == NB - 1:
                            stage(rn[:, KH, :], 1, wc)
                            stage(pn[:, KH, :], 5, wc)
                    for wc in range(2):
                        vn = sb.tile([128, KH, Z], F16, tag="vb",
                                     name=_nm("vn"))
                        stencil_apply(pn2[wc], pn2[1 - wc], wc, h0, KH, vn)
                        r0b = load_win(r0_d, wc, KH, h0, "r0b", bufs=1)
                        scr2 = sb.tile([128, KH + 2, Z], F16, tag="scr",
                                       name=_nm("scr"))
                        accD = ttr(r0b[:], vn[:], accD, scr2[:, 0:KH, :], "accD")
                        store_blk(v_dst, vn[:], wc, KH, h0 + 1)
                        if j == 0:
                            stage(vn[:, 0, :], 2, wc)
                        if j == NB - 1:
                            stage(vn[:, KH - 1, :], 3, wc)
                if not last:
                    allgather()
                    fill_rvp(dst)
                    finish_dot(accD, din[2], 0)
                    dsb2 = allreduce(2)
                    d1_ap = dsb2[0:1, 0:1]

            if dump is not None:
                dsrc, rowoff = {
                    "r0": (r0_d, 0), "v0": (vv[0], 1), "s": (s_d, 2),
                    "r1": (rr[1], 1), "v1": (vv[1], 1), "p1": (pp[1], 1),
                    "x1d": (xx[1], 0),
                }[dump]
                for j in range(NB):
                    h0 = j * KH
                    for wc in range(2):
                        g16 = load_win(dsrc, wc, KH, h0 + rowoff, "xwb",
                                       bufs=1)
                        g32 = sb.tile([128, KH, Z], F32, tag="xo",
                                      name=_nm("g32"), bufs=1)
                        nc.vector.tensor_copy(out=g32[:], in_=g16[:])
                        store_blk(x_out, g32[:], wc, KH, h0)
            _loop.close()
            if twin:
                nc.sync.dma_start(out=dummy_out[:, :], in_=z8[:])

    nc.compile()
    return nc


# ---------------------------------------------------------------------------
# host-side wrapper
# ---------------------------------------------------------------------------
_CACHE = {}


def _mats():
    An = np.zeros((128, 128), np.float16)
    for i in range(127):
        An[i, i + 1] = -1.0 / 16.0
        An[i + 1, i] = -1.0 / 16.0
    B01 = np.zeros((128, 128), np.float16)
    B01[0, 127] = -1.0 / 16.0
    B10 = np.zeros((128, 128), np.float16)
    B10[127, 0] = -1.0 / 16.0
    In = (np.eye(128) * (-1.0 / 16.0)).astype(np.float16)
    return np.concatenate([An, B01, B10, In], axis=1)


def make_in_maps(x, b, center, HC, W, Z):
    mats = _mats()
    ZR = GROUP * NSLOT * W
    H = GROUP * HC
    in_maps = []
    for c in range(N_CORES):
        bi, s = divmod(c, GROUP)
        h0 = s * HC
        xs = np.zeros((HC + 4, W, Z), np.float16)
        lo, hi = max(0, h0 - 2), min(H, h0 + HC + 2)
        xs[lo - h0 + 2:hi - h0 + 2] = x[bi, lo:hi].astype(np.float16)
        bs = np.zeros((HC + 2, W, Z), np.float16)
        lo, hi = max(0, h0 - 1), min(H, h0 + HC + 1)
        bs[lo - h0 + 1:hi - h0 + 1] = b[bi, lo:hi].astype(np.float16)
        diag_const = bool(
            np.all(center[0, :, :, 0] == center[0, 0:1, :, 0]))
        if diag_const:
            dm = np.zeros((128, 2 * 128), np.float16)
            c0 = center[0, 0, :, 0].astype(np.float32) / 16.0
            for wc in range(2):
                np.fill_diagonal(
                    dm[:, wc * 128:(wc + 1) * 128],
                    c0[wc * 128:(wc + 1) * 128].astype(np.float16))
        else:
            dm = np.zeros((128, 2 * (HC + 2) * 128), np.float16)
            ce = np.zeros((HC + 2, W), np.float32)
            lo, hi = max(0, h0 - 1), min(H, h0 + HC + 1)
            ce[lo - h0 + 1:hi - h0 + 1] = center[0, lo:hi, :, 0] / 16.0
            for wc in range(2):
                for hh in range(HC + 2):
                    k = wc * (HC + 2) + hh
                    np.fill_diagonal(
                        dm[:, k * 128:(k + 1) * 128],
                        ce[hh, wc * 128:(wc + 1) * 128].astype(np.float16))
        idx = np.zeros((2 * 128, NSLOT), np.int32)
        w128 = np.arange(128, dtype=np.int32)
        fills = _fills_rvp(HC) + _fills_s(HC)
        for i, (_f, _row, slot, left) in enumerate(fills):
            nb = s - 1 if left else s + 1
            for wc in range(2):
                if 0 <= nb < GROUP:
                    rows = nb * NSLOT * W + slot * W + wc * 128 + w128
                else:
                    rows = ZR + w128
                idx[wc * 128:(wc + 1) * 128, i] = rows
        me = np.ones((128, 2), np.float32)
        if s == 0:
            me[:, 0] = 0.0
        if s == GROUP - 1:
            me[:, 1] = 0.0
        in_maps.append({"x16": xs, "b16": bs, "mats": mats, "dmats": dm,
                        "idx": idx, "medge": me})
    return in_maps, diag_const


RUN_WALL_S = []


def kernel(x, b, ref, center):
    import time as _time
    x = np.asarray(x)
    b = np.asarray(b)
    center = np.asarray(center)
    B, H, W, Z = x.shape
    HC = H // GROUP
    from concourse.bass_utils import run_bass_kernel_spmd
    in_maps, diag_const = make_in_maps(x, b, center, HC, W, Z)
    key = (HC, W, Z, diag_const)
    if key not in _CACHE:
        _CACHE[key] = build_program(HC=HC, W=W, Z=Z, diag_const=diag_const)
    nc = _CACHE[key]
    _t0 = _time.time()
    res = run_bass_kernel_spmd(nc, in_maps, core_ids=list(range(N_CORES)))
    RUN_WALL_S.append(_time.time() - _t0)
    out = np.empty((B, H, W, Z), np.float32)
    for c in range(N_CORES):
        bi, s = divmod(c, GROUP)
        out[bi, s * HC:(s + 1) * HC] = res.results[c]["xout"]
    return out


# revision 4
# speedup vs baseline: 2.3087x; 2.3087x over previous
"""BiCGSTAB (4 iters, 7-point stencil) on 8 NeuronCores — fp16/fused version.

Problem: x,b,ref: [2,256,256,256] f32, center: [1,256,256,1] f32.
Output: final x after 4 BiCGSTAB iterations (f32).

Sharding: core c handles batch c//4, H-slab 64*(c%4)..+64. Dot products are
4-rank AllReduces in groups [[0..3],[4..7]]; H-halo ghost planes are filled
from AllGathers via indirect-DMA gathers (edge cores read a zeroed row
range, implementing the Dirichlet boundary).

vs the f32 baseline:
- Fields stored fp16 (x,b pre-converted host-side); v stored as v/4 and the
  stencil applied with coefficients scaled by 1/4 (center reaches 262, so
  S(S(.)) overflows fp16 unscaled; sim'd rel err 1.4e-3 vs 2e-2 gate).
- The whole stencil (center via per-plane diag matrices, W/H/Z shifts) runs
  on the TensorEngine as accumulating fp16 matmuls into PSUM; ScalarE
  copies PSUM->SBUF with scale factors folded in.
- Two fused sweeps per iteration instead of 5 field passes:
    sweep1: s = r - 4a*v~; t' = S4(s); dots <t',s>,<t',t'>,<r0,t'>; store s
            (s boundary planes exchanged under the omega-AllReduce barrier).
    sweep2: recompute t' from s; r' = s - 4w*t'; p' = r' + B*(p - 4w*v~);
            v~' = S4(p'); x += a*p + w*s; dot <r0,v~'>;
            r/v/p boundary planes exchanged for the next iteration.
"""
import numpy as np

import concourse.bacc as bacc
import concourse.bass as bass
import concourse.bass_isa as bass_isa
import concourse.mybir as mybir
import concourse.tile as tile

F32 = mybir.dt.float32
F16 = mybir.dt.float16
I32 = mybir.dt.int32

N_CORES = 8
GROUP = 4
EPS = 1e-6
SC = 16.0   # stencil coefficient scale: psum = S(u)/SC
# halo_in slots: 0:r[0] 1:r[HC-1] 2:v[0] 3:v[HC-1] 4:p[0] 5:p[HC-1]
#                6:s[0] 7:s[1] 8:s[HC-2] 9:s[HC-1]
NSLOT = 10


def _fills_rvp(HC):
    """(field, buffer_row, neighbor_slot, from_left); r/v/p are [HC+2]."""
    return [
        ("r", 0, 1, True), ("r", HC + 1, 0, False),
        ("v", 0, 3, True), ("v", HC + 1, 2, False),
        ("p", 0, 5, True), ("p", HC + 1, 4, False),
    ]


def _fills_s(HC):
    """s is [HC+4] with +-2 ghosts."""
    return [
        ("s", 0, 8, True), ("s", 1, 9, True),
        ("s", HC + 2, 6, False), ("s", HC + 3, 7, False),
    ]


def build_program(HC=64, W=256, Z=256, KH=8, ITERS=4, collectives=True,
                  twin_reps=0, dump=None, dump_iters=None,
                  nodots=False, nostencil=False, noupd=False, nofill=False,
                  diag_const=True, nbufs=3):
    assert W == 256 and Z == 256
    NB = HC // KH
    RG = [list(range(GROUP)), list(range(GROUP, 2 * GROUP))]
    ZR = GROUP * NSLOT * W
    PPQ = 2  # planes per 512-col psum chunk

    twin = twin_reps > 0
    assert not (twin and collectives)
    nc = bacc.Bacc("TRN2", target_bir_lowering=False, debug=False,
                   num_devices=N_CORES)

    if twin:
        x_in = nc.dram_tensor("x16_t", [HC + 4, W, Z], F16)
        b_in = nc.dram_tensor("b16_t", [HC + 2, W, Z], F16)
        x_out = nc.dram_tensor("xout_t", [HC, W, Z], F32)
        dummy_out = nc.dram_tensor("dummy_o", [1, 8], F32,
                                   kind="ExternalOutput")
    else:
        x_in = nc.dram_tensor("x16", [HC + 4, W, Z], F16,
                              kind="ExternalInput")
        b_in = nc.dram_tensor("b16", [HC + 2, W, Z], F16,
                              kind="ExternalInput")
        x_out = nc.dram_tensor("xout", [HC, W, Z], F32, kind="ExternalOutput")
    mats_in = nc.dram_tensor("mats", [128, 4 * 128], F16, kind="ExternalInput")
    NDM = 2 if diag_const else 2 * (HC + 2)
    dmats_in = nc.dram_tensor("dmats", [128, NDM * 128], F16,
                              kind="ExternalInput")
    idx_in = nc.dram_tensor("idx", [2 * 128, NSLOT], I32, kind="ExternalInput")
    medge_in = nc.dram_tensor("medge", [128, 2], F32, kind="ExternalInput")

    with tile.TileContext(nc) as tc:
        with (
            tc.tile_pool(name="sb", bufs=2) as sb,
            tc.tile_pool(name="ps", bufs=8, space="PSUM") as ps,
            tc.tile_pool(name="dr", bufs=1, space="DRAM") as dr,
        ):
            _cnt = [0]

            def _nm(pfx):
                _cnt[0] += 1
                return f"{pfx}{_cnt[0]}"

            # ---- DRAM intermediates
            r0_d = dr.tile([HC, W, Z], F16, tag="r0")
            s_d = dr.tile([HC + 4, W, Z], F16, tag="s")
            rr = [dr.tile([HC + 2, W, Z], F16, tag=f"r{i}", name=f"fld_r{i}")
                  for i in range(2)]
            vv = [dr.tile([HC + 2, W, Z], F16, tag=f"v{i}", name=f"fld_v{i}")
                  for i in range(2)]
            pp = [dr.tile([HC + 2, W, Z], F16, tag=f"p{i}", name=f"fld_p{i}")
                  for i in range(2)]
            xx = [dr.tile([HC, W, Z], F16, tag=f"x{i}", name=f"fld_x{i}")
                  for i in range(2)]
            halo_in = dr.tile([NSLOT * W, Z], F16, tag="halo_in")
            halo_out = dr.tile([ZR + 128, Z], F16, tag="halo_out")
            din = [dr.tile([1, 8], F32, tag=f"din{i}", name=f"din_{i}")
                   for i in range(3)]
            dout = [dr.tile([1, 8], F32, tag=f"dout{i}", name=f"dout_{i}")
                    for i in range(3)]

            # ---- persistent SBUF constants
            mats_sb = sb.tile([128, 4 * 128], F16, tag="mats", bufs=1)
            nc.sync.dma_start(out=mats_sb[:], in_=mats_in[:, :])
            An = mats_sb[:, 0:128]
            Bn = [mats_sb[:, 128:256], mats_sb[:, 256:384]]
            In = mats_sb[:, 384:512]
            dmats_sb = sb.tile([128, NDM * 128], F16, tag="dmats",
                               bufs=1)
            nc.sync.dma_start(out=dmats_sb[:], in_=dmats_in[:, :])

            def Dg(wc, h):
                k = wc if diag_const else wc * (HC + 2) + h + 1
                return dmats_sb[:, k * 128:(k + 1) * 128]

            assert diag_const, ("h-varying center path needs per-plane "
                                "diag matmuls; rebuild with the v1 kernel")

            idx_sb = []
            for wc in range(2):
                it_ = sb.tile([128, NSLOT], I32, tag=f"idx{wc}", bufs=1,
                              name=f"idx_{wc}")
                nc.sync.dma_start(out=it_[:],
                                  in_=idx_in[wc * 128:(wc + 1) * 128, :])
                idx_sb.append(it_)
            medge = sb.tile([128, 2], F32, tag="medge", bufs=1)
            nc.sync.dma_start(out=medge[:], in_=medge_in[:, :])

            zt = sb.tile([128, Z], F16, tag="gh")
            nc.vector.memset(zt[:], 0.0)
            nc.sync.dma_start(out=halo_out[ZR:ZR + 128, :], in_=zt[:])
            z8 = sb.tile([1, 8], F32, tag="z8", bufs=1)
            nc.vector.memset(z8[:], 0.0)
            for i in range(3):
                nc.sync.dma_start(out=din[i][:, :], in_=z8[:])

            # ---- helpers ------------------------------------------------
            def stage(src_plane_ap, slot, wc):
                r_ = slot * W + wc * 128
                nc.sync.dma_start(out=halo_in[r_:r_ + 128, :],
                                  in_=src_plane_ap)

            def allgather():
                if collectives:
                    nc.gpsimd.collective_compute(
                        "AllGather", mybir.AluOpType.bypass, replica_groups=RG,
                        ins=[halo_in[:, :].opt()],
                        outs=[halo_out[0:ZR, :].opt()])

            def fill(fills, flds, icol0):
                if nofill:
                    return
                for i, (f, row, _s, _l) in enumerate(fills):
                    for wc in range(2):
                        g = sb.tile([128, Z], F16, tag="gh", name=_nm("gh"))
                        nc.gpsimd.indirect_dma_start(
                            out=g[:], out_offset=None, in_=halo_out[:, :],
                            in_offset=bass.IndirectOffsetOnAxis(
                                ap=idx_sb[wc][:, icol0 + i:icol0 + i + 1],
                                axis=0))
                        nc.sync.dma_start(
                            out=flds[f][row, wc * 128:wc * 128 + 128, :],
                            in_=g[:])

            def fill_rvp(si):
                fill(_fills_rvp(HC), {"r": rr[si], "v": vv[si], "p": pp[si]},
                     0)

            def fill_s():
                fill(_fills_s(HC), {"s": s_d}, 6)

            def load_win(field, wc, npl, row0, tag, bufs=None):
                w0 = wc * 128
                t_ = sb.tile([128, npl, Z], F16, tag=tag, name=_nm("w"),
                             bufs=bufs if bufs else nbufs)
                nc.sync.dma_start(
                    out=t_[:],
                    in_=field[row0:row0 + npl, w0:w0 + 128, :].rearrange(
                        "h w z -> w h z"))
                return t_

            def store_blk(field, src_ap, wc, npl, row0):
                w0 = wc * 128
                nc.sync.dma_start(
                    out=field[row0:row0 + npl, w0:w0 + 128, :].rearrange(
                        "h w z -> w h z"),
                    in_=src_ap)

            def stencil_apply(sw, so, wc, h0, npl, out_tile, mul=None,
                              combine=None):
                """out_tile = S4(field) on npl planes [h0..h0+npl-1].

                sw/so: this/other chunk windows covering planes
                [h0-1 .. h0+npl] (npl+2 planes). mul: [128,1] AP folded into
                the ScalarE PSUM->SBUF copy. combine(dst, pt, q): custom DVE
                combine instead.
                """
                nq = (npl * Z) // 512
                of = out_tile[:].rearrange("p h z -> p (h z)")
                if nostencil:
                    return
                for q in range(nq):
                    pt = ps.tile([128, 512], F32, tag="pt", name=_nm("pt"))
                    p3 = pt[:].rearrange("p (h z) -> p h z", h=PPQ)
                    qp = q * PPQ
                    # combined diag(cen/SC) - A/SC (same rhs -> one matmul)
                    nc.tensor.matmul(
                        out=pt[:], lhsT=Dg(wc, 0) if diag_const else None,
                        rhs=sw[:, 1 + qp:1 + qp + PPQ, :].rearrange(
                            "p h z -> p (h z)"),
                        start=True, stop=False)
                    nc.tensor.matmul(
                        out=pt[:], lhsT=Bn[wc],
                        rhs=so[:, 1 + qp:1 + qp + PPQ, :].rearrange(
                            "p h z -> p (h z)"),
                        start=False, stop=False)
                    nc.tensor.matmul(
                        out=pt[:], lhsT=In,
                        rhs=sw[:, qp:qp + PPQ, :].rearrange(
                            "p h z -> p (h z)"),
                        start=False, stop=False)
                    nc.tensor.matmul(
                        out=pt[:], lhsT=In,
                        rhs=sw[:, 2 + qp:2 + qp + PPQ, :].rearrange(
                            "p h z -> p (h z)"),
                        start=False, stop=False)
                    nc.tensor.matmul(
                        out=p3[:, :, 1:Z], lhsT=In,
                        rhs=sw[:, 1 + qp:1 + qp + PPQ, 0:Z - 1],
                        start=False, stop=False)
                    nc.tensor.matmul(
                        out=p3[:, :, 0:Z - 1], lhsT=In,
                        rhs=sw[:, 1 + qp:1 + qp + PPQ, 1:Z],
                        start=False, stop=True)
                    dst = of[:, q * 512:(q + 1) * 512]
                    if combine is not None:
                        combine(dst, pt, q)
                    elif mul is None:
                        nc.scalar.copy(out=dst, in_=pt[:])
                    else:
                        nc.scalar.mul(out=dst, in_=pt[:], mul=mul)

            def ttr(in0, in1, acc_prev, scr, tag="accA"):
                acc = sb.tile([128, 1], F32, tag=tag + "p", bufs=4,
                              name=_nm("acc"))
                if nodots:
                    nc.vector.memset(acc[:], 1.0)
                    return acc
                nc.vector.scalar_tensor_tensor(
                    out=scr, in0=in0, scalar=1.0, in1=in1,
                    op0=mybir.AluOpType.mult, op1=mybir.AluOpType.mult,
                    accum_out=acc[:])
                if acc_prev is None:
                    return acc
                tot = sb.tile([128, 1], F32, tag=tag, bufs=4, name=_nm("accs"))
                nc.vector.tensor_add(out=tot[:], in0=acc_prev[:], in1=acc[:])
                return tot

            def finish_dot(acc, dtile, col):
                red = sb.tile([128, 1], F32, tag="dred", bufs=8, name=_nm("rd"))
                nc.gpsimd.partition_all_reduce(
                    red[:], acc[:], channels=128,
                    reduce_op=bass_isa.ReduceOp.add)
                nc.sync.dma_start(out=dtile[0:1, col:col + 1],
                                  in_=red[0:1, 0:1])

            def allreduce(i):
                if collectives:
                    nc.gpsimd.collective_compute(
                        "AllReduce", mybir.AluOpType.add, replica_groups=RG,
                        ins=[din[i][:, :].opt()], outs=[dout[i][:, :].opt()])
                dsb = sb.tile([1, 8], F32, tag="dsb", bufs=8, name=_nm("dsb"))
                nc.sync.dma_start(out=dsb[:], in_=dout[i][:, :])
                return dsb

            def s_tile():
                return sb.tile([1, 1], F32, tag="dsc", bufs=64, name=_nm("sc"))

            def s_recip_eps(a_ap, pre=1.0):
                if pre != 1.0:
                    t2 = s_tile()
                    nc.vector.tensor_scalar_mul(out=t2[:], in0=a_ap,
                                                scalar1=pre)
                    a_ap = t2[:]
                t = s_tile()
                nc.vector.tensor_scalar_add(out=t[:], in0=a_ap, scalar1=EPS)
                r_ = s_tile()
                nc.vector.reciprocal(out=r_[:], in_=t[:])
                return r_

            def s_mul(a_ap, b_ap):
                t = s_tile()
                nc.vector.tensor_tensor(out=t[:], in0=a_ap, in1=b_ap,
                                        op=mybir.AluOpType.mult)
                return t

            def s_muli(a_ap, imm):
                t = s_tile()
                nc.vector.tensor_scalar_mul(out=t[:], in0=a_ap, scalar1=imm)
                return t

            def s_sub(a_ap, b_ap):
                t = s_tile()
                nc.vector.tensor_tensor(out=t[:], in0=a_ap, in1=b_ap,
                                        op=mybir.AluOpType.subtract)
                return t

            def bcast(a_ap):
                b_ = sb.tile([128, 1], F32, tag="bc", bufs=16, name=_nm("bc"))
                nc.gpsimd.partition_broadcast(b_[:], a_ap, channels=128)
                return b_

            def stt(out, in0, sc, in1):
                nc.vector.scalar_tensor_tensor(
                    out=out, in0=in0, scalar=sc, in1=in1,
                    op0=mybir.AluOpType.mult, op1=mybir.AluOpType.add)

            from contextlib import ExitStack as _ES
            _loop = _ES()
            if twin:
                _loop.enter_context(tc.For_i(0, twin_reps, 1))

            # ================= P0 =======================================
            accR = accD = None
            for j in range(NB):
                h0 = j * KH
                xw = [load_win(x_in, wc, KH + 4, h0, f"rw{wc}")
                      for wc in range(2)]
                bw = [load_win(b_in, wc, KH + 2, h0, f"vw{wc}")
                      for wc in range(2)]
                r0w = []
                for wc in range(2):
                    t_ = sb.tile([128, KH + 2, Z], F16, tag=f"sw{wc}",
                                 name=_nm("r0w"), bufs=nbufs)
                    bf = bw[wc][:].rearrange("p h z -> p (h z)")

                    def comb(dst, pt, q, bf=bf):
                        nc.vector.scalar_tensor_tensor(
                            out=dst, in0=pt[:], scalar=-SC,
                            in1=bf[:, q * 512:(q + 1) * 512],
                            op0=mybir.AluOpType.mult, op1=mybir.AluOpType.add)

                    stencil_apply(xw[wc], xw[1 - wc], wc, h0 - 1, KH + 2, t_,
                                  combine=comb)
                    if j == 0:
                        nc.vector.tensor_scalar_mul(
                            out=t_[:, 0, :], in0=t_[:, 0, :],
                            scalar1=medge[:, 0:1])
                    if j == NB - 1:
                        nc.vector.tensor_scalar_mul(
                            out=t_[:, KH + 1, :], in0=t_[:, KH + 1, :],
                            scalar1=medge[:, 1:2])
                    r0w.append(t_)
                for wc in range(2):
                    vb = sb.tile([128, KH, Z], F16, tag="vb", name=_nm("vb"))
                    stencil_apply(r0w[wc], r0w[1 - wc], wc, h0, KH, vb)
                    r0c = r0w[wc][:, 1:KH + 1, :]
                    scr = sb.tile([128, KH + 2, Z], F16, tag="scr",
                                  name=_nm("scr"))
                    sc8 = scr[:, 0:KH, :]
                    accR = ttr(r0c, r0c, accR, sc8, "accR")
                    accD = ttr(r0c, vb[:], accD, sc8, "accD")
                    store_blk(r0_d, r0c, wc, KH, h0)
                    store_blk(rr[0], r0c, wc, KH, h0 + 1)
                    store_blk(vv[0], vb[:], wc, KH, h0 + 1)
                    if j == 0:
                        stage(r0w[wc][:, 1, :], 0, wc)
                        stage(vb[:, 0, :], 2, wc)
                        stage(r0w[wc][:, 1, :], 4, wc)
                    if j == NB - 1:
                        stage(r0w[wc][:, KH, :], 1, wc)
                        stage(vb[:, KH - 1, :], 3, wc)
                        stage(r0w[wc][:, KH, :], 5, wc)
            allgather()
            fill_rvp(0)
            finish_dot(accR, din[0], 0)
            finish_dot(accD, din[0], 1)
            dsb0 = allreduce(0)
            rho_ap = dsb0[0:1, 0:1]
            d1_ap = dsb0[0:1, 1:2]  # <r0,v~>; <r0,v> = 4*d1

            for it in range(ITERS):
                last = it == ITERS - 1
                src, dst = it % 2, 1 - it % 2
                r_src, v_src = rr[src], vv[src]
                p_src = rr[0] if it == 0 else pp[src]
                r_dst, v_dst, p_dst = rr[dst], vv[dst], pp[dst]

                alpha = s_mul(rho_ap, s_recip_eps(d1_ap, SC)[:])
                na4 = bcast(s_muli(alpha[:], -SC)[:])
                a_b = bcast(alpha[:])

                # ===== sweep 1 =====
                accTS = accTT = accRT = None
                for j in [0, NB - 1] + list(range(1, NB - 1)):
                    h0 = j * KH
                    rw = [load_win(r_src, wc, KH + 2, h0, f"rw{wc}")
                          for wc in range(2)]
                    vw = [load_win(v_src, wc, KH + 2, h0, f"vw{wc}")
                          for wc in range(2)]
                    sw = []
                    for wc in range(2):
                        st_ = sb.tile([128, KH + 2, Z], F16, tag=f"sw{wc}",
                                      name=_nm("sw"), bufs=nbufs)
                        if not noupd:
                            stt(st_[:], vw[wc][:], na4[:], rw[wc][:])
                        sw.append(st_)
                    for wc in range(2):
                        tw = sb.tile([128, KH + 2, Z], F16, tag="tw",
                                     name=_nm("tw"), bufs=2)
                        tw8 = tw[:, 0:KH, :]
                        stencil_apply(sw[wc], sw[1 - wc], wc, h0, KH, tw8)
                        scr = sb.tile([128, KH + 2, Z], F16, tag="scr",
                                      name=_nm("scr"))
                        sc8 = scr[:, 0:KH, :]
                        sc_ = sw[wc][:, 1:KH + 1, :]
                        accTS = ttr(tw8, sc_, accTS, sc8, "accTS")
                        accTT = ttr(tw8, tw8, accTT, sc8, "accTT")
                        if not last:
                            if it == 0:
                                r0b_ap = rw[wc][:, 1:KH + 1, :]
                            else:
                                r0b_ap = load_win(r0_d, wc, KH, h0, "r0b",
                                                  bufs=1)[:]
                            accRT = ttr(r0b_ap, tw8, accRT, sc8, "accRT")
                        store_blk(s_d, sc_, wc, KH, h0 + 2)
                        if j == 0:
                            stage(sw[wc][:, 1, :], 6, wc)
                            stage(sw[wc][:, 2, :], 7, wc)
                        if j == NB - 1:
                            stage(sw[wc][:, KH - 1, :], 8, wc)
                            stage(sw[wc][:, KH, :], 9, wc)
                allgather()
                fill_s()
                finish_dot(accTS, din[1], 0)
                finish_dot(accTT, din[1], 1)
                if not last:
                    finish_dot(accRT, din[1], 2)
                dsb1 = allreduce(1)
                omega = s_mul(s_muli(dsb1[0:1, 0:1], SC)[:],
                              s_recip_eps(dsb1[0:1, 1:2], SC * SC)[:])
                o_b = bcast(omega[:])
                no4 = bcast(s_muli(omega[:], -SC)[:])
                if not last:
                    rho_n = s_sub(
                        s_sub(rho_ap,
                              s_muli(s_mul(alpha[:], d1_ap)[:], SC)[:])[:],
                        s_muli(s_mul(omega[:], dsb1[0:1, 2:3])[:], SC)[:])
                    beta = s_mul(
                        s_mul(rho_n[:], s_recip_eps(rho_ap)[:])[:],
                        s_mul(alpha[:], s_recip_eps(omega[:])[:])[:])
                    b_b = bcast(beta[:])
                    rho_ap = rho_n[:]

                # ===== sweep 2 =====
                accD = None
                for j in ([0, NB - 1] + list(range(1, NB - 1))
                          if not last else list(range(NB))):
                    h0 = j * KH
                    if last:
                        for wc in range(2):
                            sb_c = load_win(s_d, wc, KH, h0 + 2, "sw0")  # nbufs default
                            pw = load_win(p_src, wc, KH, h0 + 1, "pw", bufs=2)
                            xwb = (load_win(x_in, wc, KH, h0 + 2, "xwb",
                                            bufs=1)
                                   if it == 0 else
                                   load_win(xx[src], wc, KH, h0, "xwb",
                                            bufs=1))
                            x1 = sb.tile([128, KH, Z], F16, tag="x1",
                                         name=_nm("x1"))
                            stt(x1[:], pw[:], a_b[:], xwb[:])
                            xo = sb.tile([128, KH, Z], F32, tag="xo",
                                         name=_nm("xo"), bufs=1)
                            stt(xo[:], sb_c[:], o_b[:], x1[:])
                            store_blk(x_out, xo[:], wc, KH, h0)
                        continue
                    sw4 = [load_win(s_d, wc, KH + 4, h0, f"rw{wc}")
                           for wc in range(2)]
                    pn2 = []
                    for wc in range(2):
                        twm = sb.tile([128, KH + 2, Z], F16, tag="tw",
                                      name=_nm("twm"), bufs=2)
                        stencil_apply(sw4[wc], sw4[1 - wc], wc, h0 - 1,
                                      KH + 2, twm, mul=no4[:, 0:1])
                        rn = sb.tile([128, KH + 2, Z], F16, tag="rn",
                                     name=_nm("rn"))
                        if not noupd:
                            nc.vector.tensor_add(out=rn[:],
                                                 in0=sw4[wc][:, 1:KH + 3, :],
                                                 in1=twm[:])
                        vw2 = load_win(v_src, wc, KH + 2, h0, f"vw{wc}")
                        pw = load_win(p_src, wc, KH + 2, h0, "pw", bufs=2)
                        u = sb.tile([128, KH + 2, Z], F16, tag="scr",
                                    name=_nm("u"))
                        pn = sb.tile([128, KH + 2, Z], F16, tag=f"pn{wc}",
                                     name=_nm("pn"), bufs=2)
                        if not noupd:
                            stt(u[:], vw2[:], no4[:], pw[:])
                            stt(pn[:], u[:], b_b[:], rn[:])
                        pn2.append(pn)
                        # x update
                        xwb = (load_win(x_in, wc, KH, h0 + 2, "xwb", bufs=1)
                               if it == 0 else
                               load_win(xx[src], wc, KH, h0, "xwb", bufs=1))
                        x1 = sb.tile([128, KH, Z], F16, tag="x1",
                                     name=_nm("x1"))
                        x2 = sb.tile([128, KH, Z], F16, tag="x2",
                                     name=_nm("x2"))
                        if not noupd:
                            stt(x1[:], pw[:, 1:KH + 1, :], a_b[:], xwb[:])
                            stt(x2[:], sw4[wc][:, 2:KH + 2, :], o_b[:], x1[:])
                        store_blk(xx[dst], x2[:], wc, KH, h0)
                        store_blk(r_dst, rn[:, 1:KH + 1, :], wc, KH, h0 + 1)
                        store_blk(p_dst, pn[:, 1:KH + 1, :], wc, KH, h0 + 1)
                        if j == 0:
                            stage(rn[:, 1, :], 0, wc)
                            stage(pn[:, 1, :], 4, wc)
                        if j == NB - 1:
                            stage(rn[:, KH, :], 1, wc)
                            stage(pn[:, KH, :], 5, wc)
                    for wc in range(2):
                        vn = sb.tile([128, KH, Z], F16, tag="vb",
                                     name=_nm("vn"))
                        stencil_apply(pn2[wc], pn2[1 - wc], wc, h0, KH, vn)
                        r0b = load_win(r0_d, wc, KH, h0, "r0b", bufs=1)
                        scr2 = sb.tile([128, KH + 2, Z], F16, tag="scr",
                                       name=_nm("scr"))
                        accD = ttr(r0b[:], vn[:], accD, scr2[:, 0:KH, :], "accD")
                        store_blk(v_dst, vn[:], wc, KH, h0 + 1)
                        if j == 0:
                            stage(vn[:, 0, :], 2, wc)
                        if j == NB - 1:
                            stage(vn[:, KH - 1, :], 3, wc)
                if not last:
                    allgather()
                    fill_rvp(dst)
                    finish_dot(accD, din[2], 0)
                    dsb2 = allreduce(2)
                    d1_ap = dsb2[0:1, 0:1]

            if dump is not None:
                dsrc, rowoff = {
                    "r0": (r0_d, 0), "v0": (vv[0], 1), "s": (s_d, 2),
                    "r1": (rr[1], 1), "v1": (vv[1], 1), "p1": (pp[1], 1),
                    "x1d": (xx[1], 0),
                }[dump]
                for j in range(NB):
                    h0 = j * KH
                    for wc in range(2):
                        g16 = load_win(dsrc, wc, KH, h0 + rowoff, "xwb",
                                       bufs=1)
                        g32 = sb.tile([128, KH, Z], F32, tag="xo",
                                      name=_nm("g32"), bufs=1)
                        nc.vector.tensor_copy(out=g32[:], in_=g16[:])
                        store_blk(x_out, g32[:], wc, KH, h0)
            _loop.close()
            if twin:
                nc.sync.dma_start(out=dummy_out[:, :], in_=z8[:])

    nc.compile()
    return nc


# ---------------------------------------------------------------------------
# host-side wrapper
# ---------------------------------------------------------------------------
_CACHE = {}


def _mats():
    An = np.zeros((128, 128), np.float16)
    for i in range(127):
        An[i, i + 1] = -1.0 / 16.0
        An[i + 1, i] = -1.0 / 16.0
    B01 = np.zeros((128, 128), np.float16)
    B01[0, 127] = -1.0 / 16.0
    B10 = np.zeros((128, 128), np.float16)
    B10[127, 0] = -1.0 / 16.0
    In = (np.eye(128) * (-1.0 / 16.0)).astype(np.float16)
    return np.concatenate([An, B01, B10, In], axis=1)


def make_in_maps(x, b, center, HC, W, Z):
    mats = _mats()
    ZR = GROUP * NSLOT * W
    H = GROUP * HC
    in_maps = []
    for c in range(N_CORES):
        bi, s = divmod(c, GROUP)
        h0 = s * HC
        xs = np.zeros((HC + 4, W, Z), np.float16)
        lo, hi = max(0, h0 - 2), min(H, h0 + HC + 2)
        xs[lo - h0 + 2:hi - h0 + 2] = x[bi, lo:hi].astype(np.float16)
        bs = np.zeros((HC + 2, W, Z), np.float16)
        lo, hi = max(0, h0 - 1), min(H, h0 + HC + 1)
        bs[lo - h0 + 1:hi - h0 + 1] = b[bi, lo:hi].astype(np.float16)
        diag_const = bool(
            np.all(center[0, :, :, 0] == center[0, 0:1, :, 0]))
        if diag_const:
            dm = np.zeros((128, 2 * 128), np.float16)
            c0 = center[0, 0, :, 0].astype(np.float32) / 16.0
            An_np = np.zeros((128, 128), np.float16)
            for i in range(127):
                An_np[i, i + 1] = -1.0 / 16.0
                An_np[i + 1, i] = -1.0 / 16.0
            for wc in range(2):
                blk = dm[:, wc * 128:(wc + 1) * 128]
                blk += An_np
                np.fill_diagonal(blk,
                                 c0[wc * 128:(wc + 1) * 128]
                                 .astype(np.float16))
        else:
            dm = np.zeros((128, 2 * (HC + 2) * 128), np.float16)
            ce = np.zeros((HC + 2, W), np.float32)
            lo, hi = max(0, h0 - 1), min(H, h0 + HC + 1)
            ce[lo - h0 + 1:hi - h0 + 1] = center[0, lo:hi, :, 0] / 16.0
            for wc in range(2):
                for hh in range(HC + 2):
                    k = wc * (HC + 2) + hh
                    np.fill_diagonal(
                        dm[:, k * 128:(k + 1) * 128],
                        ce[hh, wc * 128:(wc + 1) * 128].astype(np.float16))
        idx = np.zeros((2 * 128, NSLOT), np.int32)
        w128 = np.arange(128, dtype=np.int32)
        fills = _fills_rvp(HC) + _fills_s(HC)
        for i, (_f, _row, slot, left) in enumerate(fills):
            nb = s - 1 if left else s + 1
            for wc in range(2):
                if 0 <= nb < GROUP:
                    rows = nb * NSLOT * W + slot * W + wc * 128 + w128
                else:
                    rows = ZR + w128
                idx[wc * 128:(wc + 1) * 128, i] = rows
        me = np.ones((128, 2), np.float32)
        if s == 0:
            me[:, 0] = 0.0
        if s == GROUP - 1:
            me[:, 1] = 0.0
        in_maps.append({"x16": xs, "b16": bs, "mats": mats, "dmats": dm,
                        "idx": idx, "medge": me})
    return in_maps, diag_const


RUN_WALL_S = []


def kernel(x, b, ref, center):
    import time as _time
    x = np.asarray(x)
    b = np.asarray(b)
    center = np.asarray(center)
    B, H, W, Z = x.shape
    HC = H // GROUP
    from concourse.bass_utils import run_bass_kernel_spmd
    in_maps, diag_const = make_in_maps(x, b, center, HC, W, Z)
    key = (HC, W, Z, diag_const)
    if key not in _CACHE:
        _CACHE[key] = build_program(HC=HC, W=W, Z=Z, diag_const=diag_const)
    nc = _CACHE[key]
    _t0 = _time.time()
    res = run_bass_kernel_spmd(nc, in_maps, core_ids=list(range(N_CORES)))
    RUN_WALL_S.append(_time.time() - _t0)
    out = np.empty((B, H, W, Z), np.float32)
    for c in range(N_CORES):
        bi, s = divmod(c, GROUP)
        out[bi, s * HC:(s + 1) * HC] = res.results[c]["xout"]
    return out
